# revision 1
# baseline (speedup 1.0000x reference)
"""GCN layer on 8 trn2 cores.

Math: out = segment_sum((h@W * norm)[src], dst) * norm + bias
Linearity reorder: out = (segment_sum((h*norm)[src], dst) @ W) * norm + bias
=> aggregate input features first (partitioned by dst, no cross-core comm),
   GEMM + epilogue per dst shard afterwards.
"""
import numpy as np
from contextlib import ExitStack

import concourse.bass as bass
import concourse.bacc as bacc
import concourse.mybir as mybir
import concourse.tile as tile
from concourse.masks import make_identity
from concourse.bass_utils import run_bass_kernel_spmd

P = 128
N = 10000
D = 512
NCORES = 8
NPAD = 10240            # N padded to multiple of 128*NCORES
NPC = NPAD // NCORES    # nodes per core = 1280
NBLK = NPC // P         # dst blocks per core = 10
KC = D // P             # feature chunks = 4


def _build(C):
    """Build the single SPMD Bass program. C = edge chunks per dst block."""
    nc = bacc.Bacc(None, target_bir_lowering=False)
    f32 = mybir.dt.float32
    bf16 = mybir.dt.bfloat16
    i32 = mybir.dt.int32

    table = nc.declare_dram_parameter("table", [NPAD, D], bf16, isOutput=False)
    srci = nc.declare_dram_parameter("srci", [NBLK, P, C], i32, isOutput=False)
    rel = nc.declare_dram_parameter("rel", [NBLK, P, C], f32, isOutput=False)
    wt = nc.declare_dram_parameter("wt", [KC, P, D], f32, isOutput=False)
    nrm = nc.declare_dram_parameter("nrm", [NPC, 1], f32, isOutput=False)
    bi = nc.declare_dram_parameter("bi", [P, D], f32, isOutput=False)
    iota = nc.declare_dram_parameter("iota", [P, P], f32, isOutput=False)
    out = nc.declare_dram_parameter("out", [NPC, D], f32, isOutput=True)

    with tile.TileContext(nc) as tc, ExitStack() as ctx:
        const = ctx.enter_context(tc.tile_pool(name="const", bufs=1))
        epool = ctx.enter_context(tc.tile_pool(name="edges", bufs=NBLK))
        gpool = ctx.enter_context(tc.tile_pool(name="gath", bufs=8))
        spool = ctx.enter_context(tc.tile_pool(name="sel", bufs=8))
        apool = ctx.enter_context(tc.tile_pool(name="accs", bufs=NBLK))
        tpool = ctx.enter_context(tc.tile_pool(name="trs", bufs=4 * NBLK))
        opool = ctx.enter_context(tc.tile_pool(name="outs", bufs=NBLK))
        ps1 = ctx.enter_context(tc.tile_pool(name="ps1", bufs=2, space="PSUM"))
        pst = ctx.enter_context(tc.tile_pool(name="pst", bufs=4, space="PSUM"))
        ps2 = ctx.enter_context(tc.tile_pool(name="ps2", bufs=2, space="PSUM"))

        iota_t = const.tile([P, P], f32)
        nc.sync.dma_start(out=iota_t[:], in_=iota[:])
        ident_t = const.tile([P, P], f32)
        make_identity(nc, ident_t[:])
        bias_t = const.tile([P, D], f32)
        nc.sync.dma_start(out=bias_t[:], in_=bi[:])
        w_t = const.tile([P, KC * D], f32)
        for kc in range(KC):
            nc.sync.dma_start(out=w_t[:, kc * D:(kc + 1) * D], in_=wt[kc])

        for b in range(NBLK):
            idx_b = epool.tile([P, C], i32)
            nc.sync.dma_start(out=idx_b[:], in_=srci[b])
            rel_b = epool.tile([P, C], f32)
            nc.sync.dma_start(out=rel_b[:], in_=rel[b])
            nrm_b = epool.tile([P, 1], f32)
            nc.sync.dma_start(out=nrm_b[:], in_=nrm[b * P:(b + 1) * P, :])

            # accD[dst, feat] = segment-sum of gathered src rows for this
            # block, accumulated in PSUM across C edge chunks.
            accD = ps1.tile([P, D], f32, space="PSUM")
            for k in range(C):
                g_t = gpool.tile([P, D], bf16)
                nc.gpsimd.indirect_dma_start(
                    out=g_t[:],
                    out_offset=None,
                    in_=table[:],
                    in_offset=bass.IndirectOffsetOnAxis(ap=idx_b[:, k:k + 1], axis=0),
                )
                # S_T[e, j] = (rel[e] == j); padded edges have rel=-1 -> all 0
                s_t = spool.tile([P, P], bf16)
                nc.vector.tensor_tensor(
                    out=s_t[:],
                    in0=rel_b[:, k:k + 1].to_broadcast([P, P]),
                    in1=iota_t[:],
                    op=mybir.AluOpType.is_equal,
                )
                nc.tensor.matmul(
                    out=accD[:],
                    lhsT=s_t[:],
                    rhs=g_t[:],
                    start=(k == 0),
                    stop=(k == C - 1),
                )

            accS = apool.tile([P, D], f32)
            nc.vector.tensor_copy(out=accS[:], in_=accD[:])

            # out_ps[dst, :] = sum_kc A_kc @ W_kc (transpose chunks for lhsT)
            out_ps = ps2.tile([P, D], f32, space="PSUM")
            for kc in range(KC):
                tps = pst.tile([P, P], f32, space="PSUM")
                nc.tensor.transpose(
                    out=tps[:], in_=accS[:, kc * P:(kc + 1) * P],
                    identity=ident_t[:])
                lhsT_kc = tpool.tile([P, P], f32)
                nc.vector.tensor_copy(out=lhsT_kc[:], in_=tps[:])
                nc.tensor.matmul(
                    out=out_ps[:],
                    lhsT=lhsT_kc[:],
                    rhs=w_t[:, kc * D:(kc + 1) * D],
                    start=(kc == 0),
                    stop=(kc == KC - 1),
                )
            out_sb = opool.tile([P, D], f32)
            nc.vector.tensor_tensor(
                out=out_sb[:], in0=out_ps[:],
                in1=nrm_b[:].to_broadcast([P, D]),
                op=mybir.AluOpType.mult,
            )
            nc.vector.tensor_tensor(
                out=out_sb[:], in0=out_sb[:], in1=bias_t[:],
                op=mybir.AluOpType.add,
            )
            nc.sync.dma_start(out=out[b * P:(b + 1) * P, :], in_=out_sb[:])
    nc.compile()
    return nc


def _prep(h, norm, weight, bias, src, dst):
    import ml_dtypes
    hn = (h * norm).astype(np.float32)
    table = np.zeros((NPAD, D), dtype=ml_dtypes.bfloat16)
    table[:N] = hn.astype(ml_dtypes.bfloat16)

    src = np.asarray(src, dtype=np.int64)
    dst = np.asarray(dst, dtype=np.int64)
    core_of = dst // NPC
    blk_of = (dst % NPC) // P

    # chunk count: max edges landing in any (core, block), ceil to 128
    counts = np.zeros((NCORES, NBLK), dtype=np.int64)
    np.add.at(counts, (core_of, blk_of), 1)
    C = max(1, int(-(-counts.max() // P)))

    srci_all = np.zeros((NCORES, NBLK, P, C), dtype=np.int32)
    rel_all = np.full((NCORES, NBLK, P, C), -1.0, dtype=np.float32)
    gkey = core_of * NBLK + blk_of
    order = np.argsort(gkey, kind="stable")
    s_sorted = src[order]
    d_sorted = dst[order]
    g_sorted = gkey[order]
    starts = np.searchsorted(g_sorted, np.arange(NCORES * NBLK))
    ends = np.searchsorted(g_sorted, np.arange(NCORES * NBLK), side="right")
    for g in range(NCORES * NBLK):
        c, b = divmod(g, NBLK)
        lo, hi = starts[g], ends[g]
        cnt = hi - lo
        if cnt == 0:
            continue
        j = np.arange(cnt)
        srci_all[c, b, j % P, j // P] = s_sorted[lo:hi]
        rel_all[c, b, j % P, j // P] = (d_sorted[lo:hi] % P).astype(np.float32)

    normv = np.zeros((NPAD, 1), dtype=np.float32)
    normv[:N] = norm.astype(np.float32)
    wt = np.ascontiguousarray(weight.astype(np.float32).reshape(KC, P, D))
    bi = np.ascontiguousarray(
        np.broadcast_to(bias.astype(np.float32)[None, :], (P, D)))
    iota = np.ascontiguousarray(
        np.broadcast_to(np.arange(P, dtype=np.float32)[None, :], (P, P)))

    in_maps = []
    for c in range(NCORES):
        in_maps.append({
            "table": table,
            "srci": srci_all[c],
            "rel": rel_all[c],
            "wt": wt,
            "nrm": normv[c * NPC:(c + 1) * NPC],
            "bi": bi,
            "iota": iota,
        })
    return C, in_maps


_NC_CACHE = {}


def kernel(h, norm, weight, bias, src, dst):
    h = np.asarray(h, dtype=np.float32)
    norm = np.asarray(norm, dtype=np.float32)
    weight = np.asarray(weight, dtype=np.float32)
    bias = np.asarray(bias, dtype=np.float32)
    C, in_maps = _prep(h, norm, weight, bias, src, dst)
    nc = _NC_CACHE.get(C)
    if nc is None:
        nc = _NC_CACHE[C] = _build(C)
    res = run_bass_kernel_spmd(nc, in_maps, list(range(NCORES))).results
    out = np.concatenate(
        [np.asarray(res[c]["out"], dtype=np.float32) for c in range(NCORES)],
        axis=0)
    return out[:N]



# revision 6
# speedup vs baseline: 3.4526x; 3.4526x over previous
"""GCN layer on 8 trn2 cores.

Math: out = segment_sum((h@W * norm)[src], dst) * norm + bias
Linearity reorder: out = (segment_sum((h*norm)[src], dst) @ W) * norm + bias
=> aggregate input features first (partitioned by dst), GEMM + epilogue per
   dst shard afterwards.

Host->device traffic is the bottleneck (axon tunnel ~40MB/s), so each core
uploads only its 1/8 shard of (h*norm) plus its 1/8 of W (bf16, 1.4MB), and
the full 10.5MB table is assembled on-device with an AllGather over
NeuronLink. Constants (iota/identity/bias broadcast) are generated on-device;
the output returns as bf16.
"""
import numpy as np
from contextlib import ExitStack

import concourse.bass as bass
import concourse.bacc as bacc
import concourse.mybir as mybir
import concourse.tile as tile
from concourse.masks import make_identity
from concourse.bass_utils import run_bass_kernel_spmd

P = 128
N = 10000
D = 512
NCORES = 8
NPAD = 10240            # N padded to multiple of 128*NCORES
NPC = NPAD // NCORES    # node rows per core = 1280
WPC = D // NCORES       # weight rows per core = 64
SROWS = NPC + WPC       # uploaded shard rows per core = 1344
GROWS = SROWS * NCORES  # gathered rows = 10752
NBLK = NPC // P         # dst blocks per core = 10
KC = D // P             # feature chunks = 4


def _build(C):
    """Build the single SPMD Bass program. C = edge chunks per dst block."""
    nc = bacc.Bacc(None, target_bir_lowering=False)
    f32 = mybir.dt.float32
    bf16 = mybir.dt.bfloat16
    i32 = mybir.dt.int32

    hw = nc.declare_dram_parameter("hw", [SROWS, D], bf16, isOutput=False)
    srci = nc.declare_dram_parameter("srci", [NBLK, P, C], i32, isOutput=False)
    rel = nc.declare_dram_parameter("rel", [NBLK, P, C], f32, isOutput=False)
    nrm = nc.declare_dram_parameter("nrm", [NPC, 1], f32, isOutput=False)
    bi = nc.declare_dram_parameter("bi", [1, D], f32, isOutput=False)
    out = nc.declare_dram_parameter("out", [NPC, D], bf16, isOutput=True)

    with tile.TileContext(nc) as tc, ExitStack() as ctx:
        dram = ctx.enter_context(tc.tile_pool(name="dram", bufs=2, space="DRAM"))
        const = ctx.enter_context(tc.tile_pool(name="const", bufs=1))
        epool = ctx.enter_context(tc.tile_pool(name="edges", bufs=NBLK))
        gpool = ctx.enter_context(tc.tile_pool(name="gath", bufs=8))
        spool = ctx.enter_context(tc.tile_pool(name="sel", bufs=8))
        apool = ctx.enter_context(tc.tile_pool(name="accs", bufs=NBLK))
        tpool = ctx.enter_context(tc.tile_pool(name="trs", bufs=4 * NBLK))
        opool = ctx.enter_context(tc.tile_pool(name="outs", bufs=2 * NBLK))
        ps1 = ctx.enter_context(tc.tile_pool(name="ps1", bufs=2, space="PSUM"))
        pst = ctx.enter_context(tc.tile_pool(name="pst", bufs=4, space="PSUM"))
        ps2 = ctx.enter_context(tc.tile_pool(name="ps2", bufs=2, space="PSUM"))

        # Assemble the full (h*norm | W) table on-device: 1.4MB up per core,
        # AllGather does the other 9.4MB over NeuronLink.
        hwb = dram.tile([SROWS, D], bf16)
        gat = dram.tile([GROWS, D], bf16)
        nc.gpsimd.dma_start(out=hwb[:], in_=hw[:])
        nc.gpsimd.collective_compute(
            "AllGather",
            mybir.AluOpType.bypass,
            replica_groups=[list(range(NCORES))],
            ins=[hwb.opt()],
            outs=[gat.opt()],
        )

        iota_t = const.tile([P, P], f32)
        nc.gpsimd.iota(iota_t[:], [[1, P]], channel_multiplier=0,
                       allow_small_or_imprecise_dtypes=True)
        ident_t = const.tile([P, P], f32)
        make_identity(nc, ident_t[:])

        # W chunk kc lives in gathered rows of cores 2kc and 2kc+1.
        w_t = const.tile([P, KC * D], bf16)
        for c in range(NCORES):
            kc, half = divmod(c, 2)
            r0 = c * SROWS + NPC
            nc.sync.dma_start(
                out=w_t[half * WPC:(half + 1) * WPC, kc * D:(kc + 1) * D],
                in_=gat[r0:r0 + WPC, :])

        # bias broadcast [1,D] -> [P,D]: stride-0 DMA re-reads the same row
        bias_t = const.tile([P, D], f32)
        nc.sync.dma_start(out=bias_t[:], in_=bi[0:1, :].to_broadcast([P, D]))

        for b in range(NBLK):
            idx_b = epool.tile([P, C], i32)
            nc.sync.dma_start(out=idx_b[:], in_=srci[b])
            rel_b = epool.tile([P, C], f32)
            nc.sync.dma_start(out=rel_b[:], in_=rel[b])
            nrm_b = epool.tile([P, 1], f32)
            nc.sync.dma_start(out=nrm_b[:], in_=nrm[b * P:(b + 1) * P, :])

            # accD[dst, feat] = segment-sum of gathered src rows for this
            # block, accumulated in PSUM across C edge chunks.
            accD = ps1.tile([P, D], f32, space="PSUM")
            for k in range(C):
                g_t = gpool.tile([P, D], bf16)
                nc.gpsimd.indirect_dma_start(
                    out=g_t[:],
                    out_offset=None,
                    in_=gat[:],
                    in_offset=bass.IndirectOffsetOnAxis(ap=idx_b[:, k:k + 1], axis=0),
                )
                # S_T[e, j] = (rel[e] == j); padded edges have rel=-1 -> all 0
                s_t = spool.tile([P, P], bf16)
                nc.vector.tensor_tensor(
                    out=s_t[:],
                    in0=rel_b[:, k:k + 1].to_broadcast([P, P]),
                    in1=iota_t[:],
                    op=mybir.AluOpType.is_equal,
                )
                nc.tensor.matmul(
                    out=accD[:],
                    lhsT=s_t[:],
                    rhs=g_t[:],
                    start=(k == 0),
                    stop=(k == C - 1),
                )

            accS = apool.tile([P, D], f32)
            nc.vector.tensor_copy(out=accS[:], in_=accD[:])

            # out_ps[dst, :] = sum_kc A_kc @ W_kc (transpose chunks for lhsT)
            out_ps = ps2.tile([P, D], f32, space="PSUM")
            for kc in range(KC):
                tps = pst.tile([P, P], f32, space="PSUM")
                nc.tensor.transpose(
                    out=tps[:], in_=accS[:, kc * P:(kc + 1) * P],
                    identity=ident_t[:])
                lhsT_kc = tpool.tile([P, P], bf16)
                nc.vector.tensor_copy(out=lhsT_kc[:], in_=tps[:])
                nc.tensor.matmul(
                    out=out_ps[:],
                    lhsT=lhsT_kc[:],
                    rhs=w_t[:, kc * D:(kc + 1) * D],
                    start=(kc == 0),
                    stop=(kc == KC - 1),
                )
            tmp_sb = opool.tile([P, D], f32)
            nc.vector.tensor_tensor(
                out=tmp_sb[:], in0=out_ps[:],
                in1=nrm_b[:].to_broadcast([P, D]),
                op=mybir.AluOpType.mult,
            )
            out_sb = opool.tile([P, D], bf16)
            nc.vector.tensor_tensor(
                out=out_sb[:], in0=tmp_sb[:], in1=bias_t[:],
                op=mybir.AluOpType.add,
            )
            nc.sync.dma_start(out=out[b * P:(b + 1) * P, :], in_=out_sb[:])
    nc.compile()
    return nc


def _prep(h, norm, weight, bias, src, dst):
    import ml_dtypes
    hn = (h * norm).astype(np.float32)

    # per-core upload: rows 0..NPC-1 = this core's (h*norm) shard,
    # rows NPC..SROWS-1 = this core's slice of W. All bf16.
    shards = np.zeros((NCORES, SROWS, D), dtype=ml_dtypes.bfloat16)
    hn_pad = np.zeros((NPAD, D), dtype=np.float32)
    hn_pad[:N] = hn
    w32 = weight.astype(np.float32)
    for c in range(NCORES):
        shards[c, :NPC] = hn_pad[c * NPC:(c + 1) * NPC].astype(ml_dtypes.bfloat16)
        shards[c, NPC:] = w32[c * WPC:(c + 1) * WPC].astype(ml_dtypes.bfloat16)

    src = np.asarray(src, dtype=np.int64)
    dst = np.asarray(dst, dtype=np.int64)
    core_of = dst // NPC
    blk_of = (dst % NPC) // P

    # node n lives at gathered row (n // NPC) * SROWS + (n % NPC)
    src_row = (src // NPC) * SROWS + (src % NPC)

    # chunk count: max edges landing in any (core, block), ceil to 128
    counts = np.zeros((NCORES, NBLK), dtype=np.int64)
    np.add.at(counts, (core_of, blk_of), 1)
    C = max(1, int(-(-counts.max() // P)))

    srci_all = np.zeros((NCORES, NBLK, P, C), dtype=np.int32)
    rel_all = np.full((NCORES, NBLK, P, C), -1.0, dtype=np.float32)
    gkey = core_of * NBLK + blk_of
    order = np.argsort(gkey, kind="stable")
    s_sorted = src_row[order]
    d_sorted = dst[order]
    g_sorted = gkey[order]
    starts = np.searchsorted(g_sorted, np.arange(NCORES * NBLK))
    ends = np.searchsorted(g_sorted, np.arange(NCORES * NBLK), side="right")
    for g in range(NCORES * NBLK):
        c, b = divmod(g, NBLK)
        lo, hi = starts[g], ends[g]
        cnt = hi - lo
        if cnt == 0:
            continue
        j = np.arange(cnt)
        srci_all[c, b, j % P, j // P] = s_sorted[lo:hi]
        rel_all[c, b, j % P, j // P] = (d_sorted[lo:hi] % P).astype(np.float32)

    normv = np.zeros((NPAD, 1), dtype=np.float32)
    normv[:N] = norm.astype(np.float32)
    bi = np.ascontiguousarray(bias.astype(np.float32)[None, :])

    in_maps = []
    for c in range(NCORES):
        in_maps.append({
            "hw": shards[c],
            "srci": srci_all[c],
            "rel": rel_all[c],
            "nrm": normv[c * NPC:(c + 1) * NPC],
            "bi": bi,
        })
    return C, in_maps


_NC_CACHE = {}


def kernel(h, norm, weight, bias, src, dst):
    h = np.asarray(h, dtype=np.float32)
    norm = np.asarray(norm, dtype=np.float32)
    weight = np.asarray(weight, dtype=np.float32)
    bias = np.asarray(bias, dtype=np.float32)
    C, in_maps = _prep(h, norm, weight, bias, src, dst)
    nc = _NC_CACHE.get(C)
    if nc is None:
        nc = _NC_CACHE[C] = _build(C)
    res = run_bass_kernel_spmd(nc, in_maps, list(range(NCORES))).results
    out = np.concatenate(
        [np.asarray(res[c]["out"]).astype(np.float32) for c in range(NCORES)],
        axis=0)
    return out[:N]


# revision 10
# speedup vs baseline: 4.4975x; 1.3026x over previous
"""GCN layer on 8 trn2 cores.

Math: out = segment_sum((h@W * norm)[src], dst) * norm + bias
Linearity reorder: out = (segment_sum((h*norm)[src], dst) @ W) * norm + bias
=> aggregate input features first (partitioned by dst), GEMM + epilogue per
   dst shard afterwards.

Host->device traffic is the bottleneck (axon tunnel ~40-60MB/s), so:
- each core uploads only its 1/8 shard of (h*norm) plus its 1/8 of W (bf16),
  and the full table is assembled on-device with an AllGather over NeuronLink
- edge indices go up as int16, dst-slot ids as bf16
- the output returns int8-quantized (offset-128 uint8) with per-row f32
  scales, dequantized on host
- constants (iota/identity/bias broadcast) are generated on-device
- jax persistent compilation cache avoids the per-call BIR->NEFF rebuild
"""
import os
import numpy as np
from contextlib import ExitStack

import jax
jax.config.update("jax_compilation_cache_dir",
                  os.environ.get("KERNEL_JAX_CACHE", "/tmp/jax_cache"))
jax.config.update("jax_persistent_cache_min_compile_time_secs", 0)
jax.config.update("jax_persistent_cache_min_entry_size_bytes", 0)

import concourse.bass as bass
import concourse.bacc as bacc
import concourse.mybir as mybir
import concourse.tile as tile
from concourse.masks import make_identity
from concourse.bass_utils import run_bass_kernel_spmd

P = 128
N = 10000
D = 512
NCORES = 8
NPAD = 10240            # N padded to multiple of 128*NCORES
NPC = NPAD // NCORES    # node rows per core = 1280
WPC = D // NCORES       # weight rows per core = 64
SROWS = NPC + WPC       # uploaded shard rows per core = 1344
GROWS = SROWS * NCORES  # gathered rows = 10752
NBLK = NPC // P         # dst blocks per core = 10
KC = D // P             # feature chunks = 4


def _build(C):
    """Build the single SPMD Bass program. C = edge chunks per dst block."""
    nc = bacc.Bacc(None, target_bir_lowering=False)
    f32 = mybir.dt.float32
    bf16 = mybir.dt.bfloat16
    i32 = mybir.dt.int32
    i16 = mybir.dt.int16
    u8 = mybir.dt.uint8

    hw = nc.declare_dram_parameter("hw", [SROWS, D], bf16, isOutput=False)
    srci = nc.declare_dram_parameter("srci", [NBLK, P, C], i16, isOutput=False)
    rel = nc.declare_dram_parameter("rel", [NBLK, P, C], bf16, isOutput=False)
    nrm = nc.declare_dram_parameter("nrm", [NPC, 1], f32, isOutput=False)
    bi = nc.declare_dram_parameter("bi", [1, D], f32, isOutput=False)
    out = nc.declare_dram_parameter("out", [NPC, D], u8, isOutput=True)
    osc = nc.declare_dram_parameter("osc", [NPC, 1], f32, isOutput=True)

    with tile.TileContext(nc) as tc, ExitStack() as ctx:
        dram = ctx.enter_context(tc.tile_pool(name="dram", bufs=2, space="DRAM"))
        const = ctx.enter_context(tc.tile_pool(name="const", bufs=1))
        epool = ctx.enter_context(tc.tile_pool(name="edges", bufs=NBLK))
        gpool = ctx.enter_context(tc.tile_pool(name="gath", bufs=8))
        spool = ctx.enter_context(tc.tile_pool(name="sel", bufs=8))
        apool = ctx.enter_context(tc.tile_pool(name="accs", bufs=NBLK))
        tpool = ctx.enter_context(tc.tile_pool(name="trs", bufs=4 * NBLK))
        opool = ctx.enter_context(tc.tile_pool(name="outs", bufs=2 * NBLK))
        ps1 = ctx.enter_context(tc.tile_pool(name="ps1", bufs=2, space="PSUM"))
        pst = ctx.enter_context(tc.tile_pool(name="pst", bufs=4, space="PSUM"))
        ps2 = ctx.enter_context(tc.tile_pool(name="ps2", bufs=2, space="PSUM"))

        # Assemble the full (h*norm | W) table on-device: 1.4MB up per core,
        # AllGather does the other 9.4MB over NeuronLink.
        hwb = dram.tile([SROWS, D], bf16)
        gat = dram.tile([GROWS, D], bf16)
        nc.gpsimd.dma_start(out=hwb[:], in_=hw[:])
        nc.gpsimd.collective_compute(
            "AllGather",
            mybir.AluOpType.bypass,
            replica_groups=[list(range(NCORES))],
            ins=[hwb.opt()],
            outs=[gat.opt()],
        )

        iota_t = const.tile([P, P], bf16)
        nc.gpsimd.iota(iota_t[:], [[1, P]], channel_multiplier=0,
                       allow_small_or_imprecise_dtypes=True)
        ident_t = const.tile([P, P], f32)
        make_identity(nc, ident_t[:])

        # W chunk kc lives in gathered rows of cores 2kc and 2kc+1.
        w_t = const.tile([P, KC * D], bf16)
        for c in range(NCORES):
            kc, half = divmod(c, 2)
            r0 = c * SROWS + NPC
            nc.sync.dma_start(
                out=w_t[half * WPC:(half + 1) * WPC, kc * D:(kc + 1) * D],
                in_=gat[r0:r0 + WPC, :])

        # bias broadcast [1,D] -> [P,D]: stride-0 DMA re-reads the same row
        bias_t = const.tile([P, D], f32)
        nc.sync.dma_start(out=bias_t[:], in_=bi[0:1, :].to_broadcast([P, D]))

        for b in range(NBLK):
            idx16 = epool.tile([P, C], i16)
            nc.sync.dma_start(out=idx16[:], in_=srci[b])
            idx_b = epool.tile([P, C], i32)
            nc.vector.tensor_copy(out=idx_b[:], in_=idx16[:])
            rel_b = epool.tile([P, C], bf16)
            nc.sync.dma_start(out=rel_b[:], in_=rel[b])
            nrm_b = epool.tile([P, 1], f32)
            nc.sync.dma_start(out=nrm_b[:], in_=nrm[b * P:(b + 1) * P, :])

            # accD[dst, feat] = segment-sum of gathered src rows for this
            # block, accumulated in PSUM across C edge chunks.
            accD = ps1.tile([P, D], f32, space="PSUM")
            for k in range(C):
                g_t = gpool.tile([P, D], bf16)
                nc.gpsimd.indirect_dma_start(
                    out=g_t[:],
                    out_offset=None,
                    in_=gat[:],
                    in_offset=bass.IndirectOffsetOnAxis(ap=idx_b[:, k:k + 1], axis=0),
                )
                # S_T[e, j] = (rel[e] == j); padded edges have rel=-1 -> all 0
                s_t = spool.tile([P, P], bf16)
                nc.vector.tensor_tensor(
                    out=s_t[:],
                    in0=rel_b[:, k:k + 1].to_broadcast([P, P]),
                    in1=iota_t[:],
                    op=mybir.AluOpType.is_equal,
                )
                nc.tensor.matmul(
                    out=accD[:],
                    lhsT=s_t[:],
                    rhs=g_t[:],
                    start=(k == 0),
                    stop=(k == C - 1),
                )

            accS = apool.tile([P, D], f32)
            nc.vector.tensor_copy(out=accS[:], in_=accD[:])

            # out_ps[dst, :] = sum_kc A_kc @ W_kc (transpose chunks for lhsT)
            out_ps = ps2.tile([P, D], f32, space="PSUM")
            for kc in range(KC):
                tps = pst.tile([P, P], f32, space="PSUM")
                nc.tensor.transpose(
                    out=tps[:], in_=accS[:, kc * P:(kc + 1) * P],
                    identity=ident_t[:])
                lhsT_kc = tpool.tile([P, P], bf16)
                nc.vector.tensor_copy(out=lhsT_kc[:], in_=tps[:])
                nc.tensor.matmul(
                    out=out_ps[:],
                    lhsT=lhsT_kc[:],
                    rhs=w_t[:, kc * D:(kc + 1) * D],
                    start=(kc == 0),
                    stop=(kc == KC - 1),
                )
            out_f = opool.tile([P, D], f32)
            nc.vector.tensor_tensor(
                out=out_f[:], in0=out_ps[:],
                in1=nrm_b[:].to_broadcast([P, D]),
                op=mybir.AluOpType.mult,
            )
            nc.vector.tensor_tensor(
                out=out_f[:], in0=out_f[:], in1=bias_t[:],
                op=mybir.AluOpType.add,
            )
            # int8 quantization: q = round(out/rmax*127)+128 as uint8,
            # rmax returned per row for host-side dequant.
            rmax = opool.tile([P, 1], f32)
            nc.vector.tensor_reduce(
                out=rmax[:], in_=out_f[:], axis=mybir.AxisListType.X,
                op=mybir.AluOpType.max, apply_absolute_value=True)
            nc.vector.tensor_scalar_max(out=rmax[:], in0=rmax[:], scalar1=1e-20)
            rinv = opool.tile([P, 1], f32)
            nc.vector.reciprocal(out=rinv[:], in_=rmax[:])
            v_t = opool.tile([P, D], f32)
            nc.vector.tensor_tensor(
                out=v_t[:], in0=out_f[:],
                in1=rinv[:].to_broadcast([P, D]),
                op=mybir.AluOpType.mult,
            )
            q_t = opool.tile([P, D], u8)
            nc.vector.tensor_scalar(
                out=q_t[:], in0=v_t[:], scalar1=127.0, scalar2=128.5,
                op0=mybir.AluOpType.mult, op1=mybir.AluOpType.add,
            )
            nc.sync.dma_start(out=out[b * P:(b + 1) * P, :], in_=q_t[:])
            nc.sync.dma_start(out=osc[b * P:(b + 1) * P, :], in_=rinv[:])
    nc.compile()
    return nc


def _prep(h, norm, weight, bias, src, dst):
    import ml_dtypes
    hn = (h * norm).astype(np.float32)

    # per-core upload: rows 0..NPC-1 = this core's (h*norm) shard,
    # rows NPC..SROWS-1 = this core's slice of W. All bf16.
    shards = np.zeros((NCORES, SROWS, D), dtype=ml_dtypes.bfloat16)
    hn_pad = np.zeros((NPAD, D), dtype=np.float32)
    hn_pad[:N] = hn
    w32 = weight.astype(np.float32)
    for c in range(NCORES):
        shards[c, :NPC] = hn_pad[c * NPC:(c + 1) * NPC].astype(ml_dtypes.bfloat16)
        shards[c, NPC:] = w32[c * WPC:(c + 1) * WPC].astype(ml_dtypes.bfloat16)

    src = np.asarray(src, dtype=np.int64)
    dst = np.asarray(dst, dtype=np.int64)
    core_of = dst // NPC
    blk_of = (dst % NPC) // P

    # node n lives at gathered row (n // NPC) * SROWS + (n % NPC)
    src_row = (src // NPC) * SROWS + (src % NPC)

    # chunk count: max edges landing in any (core, block), ceil to 128
    counts = np.zeros((NCORES, NBLK), dtype=np.int64)
    np.add.at(counts, (core_of, blk_of), 1)
    C = max(1, int(-(-counts.max() // P)))

    srci_all = np.zeros((NCORES, NBLK, P, C), dtype=np.int16)
    rel_all = np.full((NCORES, NBLK, P, C), -1.0, dtype=ml_dtypes.bfloat16)
    gkey = core_of * NBLK + blk_of
    order = np.argsort(gkey, kind="stable")
    s_sorted = src_row[order]
    d_sorted = dst[order]
    g_sorted = gkey[order]
    starts = np.searchsorted(g_sorted, np.arange(NCORES * NBLK))
    ends = np.searchsorted(g_sorted, np.arange(NCORES * NBLK), side="right")
    for g in range(NCORES * NBLK):
        c, b = divmod(g, NBLK)
        lo, hi = starts[g], ends[g]
        cnt = hi - lo
        if cnt == 0:
            continue
        j = np.arange(cnt)
        srci_all[c, b, j % P, j // P] = s_sorted[lo:hi]
        rel_all[c, b, j % P, j // P] = (d_sorted[lo:hi] % P).astype(np.float32)

    normv = np.zeros((NPAD, 1), dtype=np.float32)
    normv[:N] = norm.astype(np.float32)
    bi = np.ascontiguousarray(bias.astype(np.float32)[None, :])

    in_maps = []
    for c in range(NCORES):
        in_maps.append({
            "hw": shards[c],
            "srci": srci_all[c],
            "rel": rel_all[c],
            "nrm": normv[c * NPC:(c + 1) * NPC],
            "bi": bi,
        })
    return C, in_maps


def _unpack(res):
    """Dequantize per-core (uint8 q, f32 1/rowmax) results to full f32 out."""
    outs = []
    for c in range(NCORES):
        q = np.asarray(res[c]["out"]).astype(np.float32)
        rinv = np.asarray(res[c]["osc"]).astype(np.float32)
        outs.append((q - 128.0) / (127.0 * rinv))
    return np.concatenate(outs, axis=0)[:N]


_NC_CACHE = {}


def kernel(h, norm, weight, bias, src, dst):
    h = np.asarray(h, dtype=np.float32)
    norm = np.asarray(norm, dtype=np.float32)
    weight = np.asarray(weight, dtype=np.float32)
    bias = np.asarray(bias, dtype=np.float32)
    C, in_maps = _prep(h, norm, weight, bias, src, dst)
    nc = _NC_CACHE.get(C)
    if nc is None:
        nc = _NC_CACHE[C] = _build(C)
    res = run_bass_kernel_spmd(nc, in_maps, list(range(NCORES))).results
    return _unpack(res)


# revision 13
# speedup vs baseline: 5.3593x; 1.1916x over previous
"""GCN layer on 8 trn2 cores.

Math: out = segment_sum((h@W * norm)[src], dst) * norm + bias
Linearity reorder: out = (segment_sum((h*norm)[src], dst) @ W) * norm + bias
=> aggregate input features first (partitioned by dst), GEMM + epilogue per
   dst shard afterwards.

Host->device traffic is the bottleneck (axon tunnel ~40-60MB/s), so:
- each core uploads only its 1/8 shard of (h*norm) int8-quantized with
  per-row bf16 scales, plus its 1/8 of W in bf16; full tables are assembled
  on-device with AllGathers over NeuronLink
- edge src ids go up as int16, dst-slot ids as bf16
- the output returns int8-quantized (offset-128 uint8) with per-row f32
  reciprocal scales, dequantized exactly on host
- constants (iota/identity/bias broadcast) are generated on-device
- jax persistent compilation cache (keyed per kernel-source hash to avoid
  stale cross-version NEFF collisions) avoids per-process recompiles
"""
import os
import hashlib
import numpy as np
from contextlib import ExitStack

import jax
with open(__file__, "rb") as _f:
    _SRC_HASH = hashlib.sha256(_f.read()).hexdigest()[:16]
jax.config.update("jax_compilation_cache_dir",
                  os.environ.get("KERNEL_JAX_CACHE",
                                 f"/tmp/jax_cache_gcn_{_SRC_HASH}"))
jax.config.update("jax_persistent_cache_min_compile_time_secs", 0)
jax.config.update("jax_persistent_cache_min_entry_size_bytes", 0)

import concourse.bass as bass
import concourse.bacc as bacc
import concourse.mybir as mybir
import concourse.tile as tile
from concourse.masks import make_identity
from concourse.bass_utils import run_bass_kernel_spmd

P = 128
N = 10000
D = 512
NCORES = 8
NPAD = 10240            # N padded to multiple of 128*NCORES
NPC = NPAD // NCORES    # node rows per core = 1280
WPC = D // NCORES       # weight rows per core = 64
NBLK = NPC // P         # dst blocks per core = 10
KC = D // P             # feature chunks = 4


def _build(C):
    """Build the single SPMD Bass program. C = edge chunks per dst block."""
    nc = bacc.Bacc(None, target_bir_lowering=False)
    f32 = mybir.dt.float32
    bf16 = mybir.dt.bfloat16
    i32 = mybir.dt.int32
    i16 = mybir.dt.int16
    i8 = mybir.dt.int8
    u8 = mybir.dt.uint8

    hq = nc.declare_dram_parameter("hq", [NPC, D], i8, isOutput=False)
    hs = nc.declare_dram_parameter("hs", [NPC, 1], f32, isOutput=False)
    wsh = nc.declare_dram_parameter("wsh", [WPC, D], bf16, isOutput=False)
    srci = nc.declare_dram_parameter("srci", [NBLK, P, C], i16, isOutput=False)
    rel = nc.declare_dram_parameter("rel", [NBLK, P, C], bf16, isOutput=False)
    nrm = nc.declare_dram_parameter("nrm", [NPC, 1], f32, isOutput=False)
    bi = nc.declare_dram_parameter("bi", [1, D], f32, isOutput=False)
    out = nc.declare_dram_parameter("out", [NPC, D], u8, isOutput=True)
    osc = nc.declare_dram_parameter("osc", [NPC, 1], f32, isOutput=True)

    with tile.TileContext(nc) as tc, ExitStack() as ctx:
        dram = ctx.enter_context(tc.tile_pool(name="dram", bufs=6, space="DRAM"))
        const = ctx.enter_context(tc.tile_pool(name="const", bufs=1))
        epool = ctx.enter_context(tc.tile_pool(name="edges", bufs=NBLK))
        gpool = ctx.enter_context(tc.tile_pool(name="gath", bufs=8))
        spool = ctx.enter_context(tc.tile_pool(name="sel", bufs=8))
        apool = ctx.enter_context(tc.tile_pool(name="accs", bufs=NBLK))
        tpool = ctx.enter_context(tc.tile_pool(name="trs", bufs=4 * NBLK))
        opool = ctx.enter_context(tc.tile_pool(name="outs", bufs=2 * NBLK))
        ps1 = ctx.enter_context(tc.tile_pool(name="ps1", bufs=2, space="PSUM"))
        pst = ctx.enter_context(tc.tile_pool(name="pst", bufs=4, space="PSUM"))
        ps2 = ctx.enter_context(tc.tile_pool(name="ps2", bufs=2, space="PSUM"))

        # Assemble full tables on-device: each core uploads 1/8, AllGather
        # moves the rest over NeuronLink.
        hqb = dram.tile([NPC, D], i8)
        hq_gat = dram.tile([NPAD, D], i8)
        nc.gpsimd.dma_start(out=hqb[:], in_=hq[:])
        nc.gpsimd.collective_compute(
            "AllGather", mybir.AluOpType.bypass,
            replica_groups=[list(range(NCORES))],
            ins=[hqb.opt()], outs=[hq_gat.opt()])

        hsb = dram.tile([NPC, 1], f32)
        hs_gat = dram.tile([NPAD, 1], f32)
        nc.gpsimd.dma_start(out=hsb[:], in_=hs[:])
        nc.gpsimd.collective_compute(
            "AllGather", mybir.AluOpType.bypass,
            replica_groups=[list(range(NCORES))],
            ins=[hsb.opt()], outs=[hs_gat.opt()])

        wb = dram.tile([WPC, D], bf16)
        w_gat = dram.tile([D, D], bf16)
        nc.gpsimd.dma_start(out=wb[:], in_=wsh[:])
        nc.gpsimd.collective_compute(
            "AllGather", mybir.AluOpType.bypass,
            replica_groups=[list(range(NCORES))],
            ins=[wb.opt()], outs=[w_gat.opt()])

        iota_t = const.tile([P, P], bf16)
        nc.gpsimd.iota(iota_t[:], [[1, P]], channel_multiplier=0,
                       allow_small_or_imprecise_dtypes=True)
        ident_t = const.tile([P, P], f32)
        make_identity(nc, ident_t[:])

        w_t = const.tile([P, KC * D], bf16)
        for kc in range(KC):
            nc.sync.dma_start(out=w_t[:, kc * D:(kc + 1) * D],
                              in_=w_gat[kc * P:(kc + 1) * P, :])

        # bias broadcast [1,D] -> [P,D]: stride-0 DMA re-reads the same row
        bias_t = const.tile([P, D], f32)
        nc.sync.dma_start(out=bias_t[:], in_=bi[0:1, :].to_broadcast([P, D]))

        for b in range(NBLK):
            idx16 = epool.tile([P, C], i16)
            nc.sync.dma_start(out=idx16[:], in_=srci[b])
            idx_b = epool.tile([P, C], i32)
            nc.vector.tensor_copy(out=idx_b[:], in_=idx16[:])
            rel_b = epool.tile([P, C], bf16)
            nc.sync.dma_start(out=rel_b[:], in_=rel[b])
            nrm_b = epool.tile([P, 1], f32)
            nc.sync.dma_start(out=nrm_b[:], in_=nrm[b * P:(b + 1) * P, :])

            # accD[dst, feat] = segment-sum of gathered src rows for this
            # block, accumulated in PSUM across C edge chunks.
            accD = ps1.tile([P, D], f32, space="PSUM")
            for k in range(C):
                gq = gpool.tile([P, D], i8)
                nc.gpsimd.indirect_dma_start(
                    out=gq[:], out_offset=None, in_=hq_gat[:],
                    in_offset=bass.IndirectOffsetOnAxis(ap=idx_b[:, k:k + 1], axis=0),
                )
                gs = gpool.tile([P, 1], f32)
                nc.gpsimd.indirect_dma_start(
                    out=gs[:], out_offset=None, in_=hs_gat[:],
                    in_offset=bass.IndirectOffsetOnAxis(ap=idx_b[:, k:k + 1], axis=0),
                )
                # dequantize: int8 row * per-row scale (exact bf16 scale)
                g_t = gpool.tile([P, D], bf16)
                nc.vector.tensor_scalar_mul(out=g_t[:], in0=gq[:], scalar1=gs[:])
                # S_T[e, j] = (rel[e] == j); padded edges have rel=-1 -> all 0
                s_t = spool.tile([P, P], bf16)
                nc.vector.tensor_tensor(
                    out=s_t[:],
                    in0=rel_b[:, k:k + 1].to_broadcast([P, P]),
                    in1=iota_t[:],
                    op=mybir.AluOpType.is_equal,
                )
                nc.tensor.matmul(
                    out=accD[:],
                    lhsT=s_t[:],
                    rhs=g_t[:],
                    start=(k == 0),
                    stop=(k == C - 1),
                )

            accS = apool.tile([P, D], f32)
            nc.vector.tensor_copy(out=accS[:], in_=accD[:])

            # out_ps[dst, :] = sum_kc A_kc @ W_kc (transpose chunks for lhsT)
            out_ps = ps2.tile([P, D], f32, space="PSUM")
            for kc in range(KC):
                tps = pst.tile([P, P], f32, space="PSUM")
                nc.tensor.transpose(
                    out=tps[:], in_=accS[:, kc * P:(kc + 1) * P],
                    identity=ident_t[:])
                lhsT_kc = tpool.tile([P, P], bf16)
                nc.vector.tensor_copy(out=lhsT_kc[:], in_=tps[:])
                nc.tensor.matmul(
                    out=out_ps[:],
                    lhsT=lhsT_kc[:],
                    rhs=w_t[:, kc * D:(kc + 1) * D],
                    start=(kc == 0),
                    stop=(kc == KC - 1),
                )
            out_f = opool.tile([P, D], f32)
            nc.vector.tensor_tensor(
                out=out_f[:], in0=out_ps[:],
                in1=nrm_b[:].to_broadcast([P, D]),
                op=mybir.AluOpType.mult,
            )
            nc.vector.tensor_tensor(
                out=out_f[:], in0=out_f[:], in1=bias_t[:],
                op=mybir.AluOpType.add,
            )
            # int8 quantization of the output, per-row reciprocal scale
            rmax = opool.tile([P, 1], f32)
            nc.vector.tensor_reduce(
                out=rmax[:], in_=out_f[:], axis=mybir.AxisListType.X,
                op=mybir.AluOpType.max, apply_absolute_value=True)
            nc.vector.tensor_scalar_max(out=rmax[:], in0=rmax[:], scalar1=1e-20)
            rinv = opool.tile([P, 1], f32)
            nc.vector.reciprocal(out=rinv[:], in_=rmax[:])
            v_t = opool.tile([P, D], f32)
            nc.vector.tensor_tensor(
                out=v_t[:], in0=out_f[:],
                in1=rinv[:].to_broadcast([P, D]),
                op=mybir.AluOpType.mult,
            )
            # HW f32->u8 cast is round-to-nearest-even with saturation, so a
            # plain +128 offset gives ideal symmetric rounding. (CoreSim
            # truncates instead, inflating sim-reported error only.)
            q_t = opool.tile([P, D], u8)
            nc.vector.tensor_scalar(
                out=q_t[:], in0=v_t[:], scalar1=127.0, scalar2=128.0,
                op0=mybir.AluOpType.mult, op1=mybir.AluOpType.add,
            )
            nc.sync.dma_start(out=out[b * P:(b + 1) * P, :], in_=q_t[:])
            nc.sync.dma_start(out=osc[b * P:(b + 1) * P, :], in_=rinv[:])
    nc.compile()
    return nc


def _prep(h, norm, weight, bias, src, dst):
    import ml_dtypes
    hn = (h * norm).astype(np.float32)
    hn_pad = np.zeros((NPAD, D), dtype=np.float32)
    hn_pad[:N] = hn

    # int8 symmetric per-row quantization with exactly-invertible f32 scale
    s = np.abs(hn_pad).max(axis=1, keepdims=True) / 127.0
    s = np.maximum(s, 1e-30).astype(np.float32)
    q = np.clip(np.rint(hn_pad / s), -127, 127).astype(np.int8)

    w16 = weight.astype(ml_dtypes.bfloat16)

    src = np.asarray(src, dtype=np.int64)
    dst = np.asarray(dst, dtype=np.int64)
    core_of = dst // NPC
    blk_of = (dst % NPC) // P

    # chunk count: max edges landing in any (core, block), ceil to 128
    counts = np.zeros((NCORES, NBLK), dtype=np.int64)
    np.add.at(counts, (core_of, blk_of), 1)
    C = max(1, int(-(-counts.max() // P)))

    srci_all = np.zeros((NCORES, NBLK, P, C), dtype=np.int16)
    rel_all = np.full((NCORES, NBLK, P, C), -1.0, dtype=ml_dtypes.bfloat16)
    gkey = core_of * NBLK + blk_of
    order = np.argsort(gkey, kind="stable")
    s_sorted = src[order]
    d_sorted = dst[order]
    g_sorted = gkey[order]
    starts = np.searchsorted(g_sorted, np.arange(NCORES * NBLK))
    ends = np.searchsorted(g_sorted, np.arange(NCORES * NBLK), side="right")
    for g in range(NCORES * NBLK):
        c, b = divmod(g, NBLK)
        lo, hi = starts[g], ends[g]
        cnt = hi - lo
        if cnt == 0:
            continue
        j = np.arange(cnt)
        srci_all[c, b, j % P, j // P] = s_sorted[lo:hi]
        rel_all[c, b, j % P, j // P] = (d_sorted[lo:hi] % P).astype(np.float32)

    normv = np.zeros((NPAD, 1), dtype=np.float32)
    normv[:N] = norm.astype(np.float32)
    bi = np.ascontiguousarray(bias.astype(np.float32)[None, :])

    in_maps = []
    for c in range(NCORES):
        in_maps.append({
            "hq": q[c * NPC:(c + 1) * NPC],
            "hs": s[c * NPC:(c + 1) * NPC],
            "wsh": np.ascontiguousarray(w16[c * WPC:(c + 1) * WPC]),
            "srci": srci_all[c],
            "rel": rel_all[c],
            "nrm": normv[c * NPC:(c + 1) * NPC],
            "bi": bi,
        })
    return C, in_maps


def _unpack(res):
    """Dequantize per-core (uint8 q, f32 1/rowmax) results to full f32 out."""
    outs = []
    for c in range(NCORES):
        q = np.asarray(res[c]["out"]).astype(np.float32)
        rinv = np.asarray(res[c]["osc"]).astype(np.float32)
        outs.append((q - 128.0) / (127.0 * rinv))
    return np.concatenate(outs, axis=0)[:N]


_NC_CACHE = {}


def kernel(h, norm, weight, bias, src, dst):
    h = np.asarray(h, dtype=np.float32)
    norm = np.asarray(norm, dtype=np.float32)
    weight = np.asarray(weight, dtype=np.float32)
    bias = np.asarray(bias, dtype=np.float32)
    C, in_maps = _prep(h, norm, weight, bias, src, dst)
    nc = _NC_CACHE.get(C)
    if nc is None:
        nc = _NC_CACHE[C] = _build(C)
    res = run_bass_kernel_spmd(nc, in_maps, list(range(NCORES))).results
    return _unpack(res)


# revision 15
# speedup vs baseline: 5.4980x; 1.0259x over previous
"""GCN layer on 8 trn2 cores.

Math: out = segment_sum((h@W * norm)[src], dst) * norm + bias
Linearity reorder: out = (segment_sum((h*norm)[src], dst) @ W) * norm + bias
=> aggregate input features first (partitioned by dst), GEMM + epilogue per
   dst shard afterwards.

Host->device traffic is the bottleneck (axon tunnel ~40-60MB/s), so:
- each core uploads only its 1/8 shard of (h*norm) int8-quantized with
  per-row bf16 scales, plus its 1/8 of W in bf16; full tables are assembled
  on-device with AllGathers over NeuronLink
- edge src ids go up as int16, dst-slot ids as bf16
- the output returns int8-quantized (offset-128 uint8) with per-row f32
  reciprocal scales, dequantized exactly on host
- constants (iota/identity/bias broadcast) are generated on-device
- jax persistent compilation cache (keyed per kernel-source hash to avoid
  stale cross-version NEFF collisions) avoids per-process recompiles
"""
import os
import hashlib
import numpy as np
from contextlib import ExitStack

import jax
with open(__file__, "rb") as _f:
    _SRC_HASH = hashlib.sha256(_f.read()).hexdigest()[:16]
jax.config.update("jax_compilation_cache_dir",
                  os.environ.get("KERNEL_JAX_CACHE",
                                 f"/tmp/jax_cache_gcn_{_SRC_HASH}"))
jax.config.update("jax_persistent_cache_min_compile_time_secs", 0)
jax.config.update("jax_persistent_cache_min_entry_size_bytes", 0)

import concourse.bass as bass
import concourse.bacc as bacc
import concourse.mybir as mybir
import concourse.tile as tile
from concourse.masks import make_identity
from concourse.bass_utils import run_bass_kernel_spmd

P = 128
N = 10000
D = 512
NCORES = 8
NPAD = 10240            # N padded to multiple of 128*NCORES
NPC = NPAD // NCORES    # node rows per core = 1280
WPC = D // NCORES       # weight rows per core = 64
NBLK = NPC // P         # dst blocks per core = 10
KC = D // P             # feature chunks = 4


def _build(C):
    """Build the single SPMD Bass program. C = edge chunks per dst block."""
    nc = bacc.Bacc(None, target_bir_lowering=False)
    f32 = mybir.dt.float32
    bf16 = mybir.dt.bfloat16
    i32 = mybir.dt.int32
    i16 = mybir.dt.int16
    i8 = mybir.dt.int8
    u8 = mybir.dt.uint8

    hq = nc.declare_dram_parameter("hq", [NPC, D], i8, isOutput=False)
    hs = nc.declare_dram_parameter("hs", [NPC, 1], f32, isOutput=False)
    wsh = nc.declare_dram_parameter("wsh", [WPC, D], bf16, isOutput=False)
    srci = nc.declare_dram_parameter("srci", [NBLK, P, C], i16, isOutput=False)
    rel = nc.declare_dram_parameter("rel", [NBLK, P, C], bf16, isOutput=False)
    nrm = nc.declare_dram_parameter("nrm", [NPC, 1], f32, isOutput=False)
    bi = nc.declare_dram_parameter("bi", [1, D], f32, isOutput=False)
    out = nc.declare_dram_parameter("out", [NPC, D], u8, isOutput=True)
    osc = nc.declare_dram_parameter("osc", [NPC, 1], f32, isOutput=True)

    with tile.TileContext(nc) as tc, ExitStack() as ctx:
        dram = ctx.enter_context(tc.tile_pool(name="dram", bufs=6, space="DRAM"))
        const = ctx.enter_context(tc.tile_pool(name="const", bufs=1))
        epool = ctx.enter_context(tc.tile_pool(name="edges", bufs=NBLK))
        gpool = ctx.enter_context(tc.tile_pool(name="gath", bufs=8))
        spool = ctx.enter_context(tc.tile_pool(name="sel", bufs=8))
        apool = ctx.enter_context(tc.tile_pool(name="accs", bufs=NBLK))
        tpool = ctx.enter_context(tc.tile_pool(name="trs", bufs=4 * NBLK))
        opool = ctx.enter_context(tc.tile_pool(name="outs", bufs=2 * NBLK))
        ps1 = ctx.enter_context(tc.tile_pool(name="ps1", bufs=2, space="PSUM"))
        pst = ctx.enter_context(tc.tile_pool(name="pst", bufs=4, space="PSUM"))
        ps2 = ctx.enter_context(tc.tile_pool(name="ps2", bufs=2, space="PSUM"))

        # Assemble full tables on-device: each core uploads 1/8, AllGather
        # moves the rest over NeuronLink.
        hqb = dram.tile([NPC, D], i8)
        hq_gat = dram.tile([NPAD, D], i8)
        nc.gpsimd.dma_start(out=hqb[:], in_=hq[:])
        nc.gpsimd.collective_compute(
            "AllGather", mybir.AluOpType.bypass,
            replica_groups=[list(range(NCORES))],
            ins=[hqb.opt()], outs=[hq_gat.opt()])

        hsb = dram.tile([NPC, 1], f32)
        hs_gat = dram.tile([NPAD, 1], f32)
        nc.gpsimd.dma_start(out=hsb[:], in_=hs[:])
        nc.gpsimd.collective_compute(
            "AllGather", mybir.AluOpType.bypass,
            replica_groups=[list(range(NCORES))],
            ins=[hsb.opt()], outs=[hs_gat.opt()])

        wb = dram.tile([WPC, D], bf16)
        w_gat = dram.tile([D, D], bf16)
        nc.gpsimd.dma_start(out=wb[:], in_=wsh[:])
        nc.gpsimd.collective_compute(
            "AllGather", mybir.AluOpType.bypass,
            replica_groups=[list(range(NCORES))],
            ins=[wb.opt()], outs=[w_gat.opt()])

        iota_t = const.tile([P, P], bf16)
        nc.gpsimd.iota(iota_t[:], [[1, P]], channel_multiplier=0,
                       allow_small_or_imprecise_dtypes=True)
        ident_t = const.tile([P, P], f32)
        make_identity(nc, ident_t[:])

        w_t = const.tile([P, KC * D], bf16)
        for kc in range(KC):
            nc.sync.dma_start(out=w_t[:, kc * D:(kc + 1) * D],
                              in_=w_gat[kc * P:(kc + 1) * P, :])

        # bias broadcast [1,D] -> [P,D]: stride-0 DMA re-reads the same row
        bias_t = const.tile([P, D], f32)
        nc.sync.dma_start(out=bias_t[:], in_=bi[0:1, :].to_broadcast([P, D]))

        for b in range(NBLK):
            idx16 = epool.tile([P, C], i16)
            nc.sync.dma_start(out=idx16[:], in_=srci[b])
            idx_b = epool.tile([P, C], i32)
            nc.vector.tensor_copy(out=idx_b[:], in_=idx16[:])
            rel_b = epool.tile([P, C], bf16)
            nc.sync.dma_start(out=rel_b[:], in_=rel[b])
            nrm_b = epool.tile([P, 1], f32)
            nc.sync.dma_start(out=nrm_b[:], in_=nrm[b * P:(b + 1) * P, :])

            # accD[dst, feat] = segment-sum of gathered src rows for this
            # block, accumulated in PSUM across C edge chunks.
            accD = ps1.tile([P, D], f32, space="PSUM")
            for k in range(C):
                gq = gpool.tile([P, D], i8)
                nc.gpsimd.indirect_dma_start(
                    out=gq[:], out_offset=None, in_=hq_gat[:],
                    in_offset=bass.IndirectOffsetOnAxis(ap=idx_b[:, k:k + 1], axis=0),
                )
                gs = gpool.tile([P, 1], f32)
                nc.gpsimd.indirect_dma_start(
                    out=gs[:], out_offset=None, in_=hs_gat[:],
                    in_offset=bass.IndirectOffsetOnAxis(ap=idx_b[:, k:k + 1], axis=0),
                )
                # dequantize: int8 row * per-row scale (exact bf16 scale)
                g_t = gpool.tile([P, D], bf16)
                nc.vector.tensor_scalar_mul(out=g_t[:], in0=gq[:], scalar1=gs[:])
                # S_T[e, j] = (rel[e] == j); padded edges have rel=-1 -> all 0
                s_t = spool.tile([P, P], bf16)
                nc.vector.tensor_tensor(
                    out=s_t[:],
                    in0=rel_b[:, k:k + 1].to_broadcast([P, P]),
                    in1=iota_t[:],
                    op=mybir.AluOpType.is_equal,
                )
                nc.tensor.matmul(
                    out=accD[:],
                    lhsT=s_t[:],
                    rhs=g_t[:],
                    start=(k == 0),
                    stop=(k == C - 1),
                )

            accS = apool.tile([P, D], f32)
            nc.vector.tensor_copy(out=accS[:], in_=accD[:])

            # out_ps[dst, :] = sum_kc A_kc @ W_kc (transpose chunks for lhsT)
            out_ps = ps2.tile([P, D], f32, space="PSUM")
            for kc in range(KC):
                tps = pst.tile([P, P], f32, space="PSUM")
                nc.tensor.transpose(
                    out=tps[:], in_=accS[:, kc * P:(kc + 1) * P],
                    identity=ident_t[:])
                lhsT_kc = tpool.tile([P, P], bf16)
                nc.vector.tensor_copy(out=lhsT_kc[:], in_=tps[:])
                nc.tensor.matmul(
                    out=out_ps[:],
                    lhsT=lhsT_kc[:],
                    rhs=w_t[:, kc * D:(kc + 1) * D],
                    start=(kc == 0),
                    stop=(kc == KC - 1),
                )
            out_f = opool.tile([P, D], f32)
            nc.vector.tensor_tensor(
                out=out_f[:], in0=out_ps[:],
                in1=nrm_b[:].to_broadcast([P, D]),
                op=mybir.AluOpType.mult,
            )
            nc.vector.tensor_tensor(
                out=out_f[:], in0=out_f[:], in1=bias_t[:],
                op=mybir.AluOpType.add,
            )
            # int8 quantization of the output, per-row reciprocal scale
            rmax = opool.tile([P, 1], f32)
            nc.vector.tensor_reduce(
                out=rmax[:], in_=out_f[:], axis=mybir.AxisListType.X,
                op=mybir.AluOpType.max, apply_absolute_value=True)
            nc.vector.tensor_scalar_max(out=rmax[:], in0=rmax[:], scalar1=1e-20)
            rinv = opool.tile([P, 1], f32)
            nc.vector.reciprocal(out=rinv[:], in_=rmax[:])
            v_t = opool.tile([P, D], f32)
            nc.vector.tensor_tensor(
                out=v_t[:], in0=out_f[:],
                in1=rinv[:].to_broadcast([P, D]),
                op=mybir.AluOpType.mult,
            )
            # HW f32->u8 cast is round-to-nearest-even with saturation, so a
            # plain +128 offset gives ideal symmetric rounding. (CoreSim
            # truncates instead, inflating sim-reported error only.)
            q_t = opool.tile([P, D], u8)
            nc.vector.tensor_scalar(
                out=q_t[:], in0=v_t[:], scalar1=127.0, scalar2=128.0,
                op0=mybir.AluOpType.mult, op1=mybir.AluOpType.add,
            )
            nc.sync.dma_start(out=out[b * P:(b + 1) * P, :], in_=q_t[:])
            nc.sync.dma_start(out=osc[b * P:(b + 1) * P, :], in_=rinv[:])
    nc.compile()
    return nc


def _prep(h, norm, weight, bias, src, dst):
    import ml_dtypes
    hn_pad = np.zeros((NPAD, D), dtype=np.float32)
    np.multiply(h, norm, out=hn_pad[:N])

    # int8 symmetric per-row quantization with exactly-invertible f32 scale
    s = np.abs(hn_pad).max(axis=1, keepdims=True)
    s /= 127.0
    np.maximum(s, 1e-30, out=s)
    np.multiply(hn_pad, 1.0 / s, out=hn_pad)
    np.rint(hn_pad, out=hn_pad)
    q = hn_pad.astype(np.int8)  # values already in [-127, 127]

    w16 = weight.astype(ml_dtypes.bfloat16)

    src = np.asarray(src, dtype=np.int64)
    dst = np.asarray(dst, dtype=np.int64)
    core_of = dst // NPC
    blk_of = (dst % NPC) // P

    # chunk count: max edges landing in any (core, block), ceil to 128
    counts = np.zeros((NCORES, NBLK), dtype=np.int64)
    np.add.at(counts, (core_of, blk_of), 1)
    C = max(1, int(-(-counts.max() // P)))

    srci_all = np.zeros((NCORES, NBLK, P, C), dtype=np.int16)
    rel_all = np.full((NCORES, NBLK, P, C), -1.0, dtype=ml_dtypes.bfloat16)
    gkey = core_of * NBLK + blk_of
    order = np.argsort(gkey, kind="stable")
    s_sorted = src[order]
    d_sorted = dst[order]
    g_sorted = gkey[order]
    starts = np.searchsorted(g_sorted, np.arange(NCORES * NBLK))
    rank = np.arange(len(g_sorted)) - starts[g_sorted]
    srci_all[g_sorted // NBLK, g_sorted % NBLK, rank % P, rank // P] = s_sorted
    rel_all[g_sorted // NBLK, g_sorted % NBLK, rank % P, rank // P] = (
        (d_sorted % P).astype(np.float32))

    normv = np.zeros((NPAD, 1), dtype=np.float32)
    normv[:N] = norm.astype(np.float32)
    bi = np.ascontiguousarray(bias.astype(np.float32)[None, :])

    in_maps = []
    for c in range(NCORES):
        in_maps.append({
            "hq": q[c * NPC:(c + 1) * NPC],
            "hs": s[c * NPC:(c + 1) * NPC],
            "wsh": np.ascontiguousarray(w16[c * WPC:(c + 1) * WPC]),
            "srci": srci_all[c],
            "rel": rel_all[c],
            "nrm": normv[c * NPC:(c + 1) * NPC],
            "bi": bi,
        })
    return C, in_maps


def _unpack(res):
    """Dequantize per-core (uint8 q, f32 1/rowmax) results to full f32 out."""
    outs = []
    for c in range(NCORES):
        q = np.asarray(res[c]["out"]).astype(np.float32)
        rinv = np.asarray(res[c]["osc"]).astype(np.float32)
        outs.append((q - 128.0) / (127.0 * rinv))
    return np.concatenate(outs, axis=0)[:N]


_NC_CACHE = {}


def kernel(h, norm, weight, bias, src, dst):
    h = np.asarray(h, dtype=np.float32)
    norm = np.asarray(norm, dtype=np.float32)
    weight = np.asarray(weight, dtype=np.float32)
    bias = np.asarray(bias, dtype=np.float32)
    C, in_maps = _prep(h, norm, weight, bias, src, dst)
    nc = _NC_CACHE.get(C)
    if nc is None:
        nc = _NC_CACHE[C] = _build(C)
    res = run_bass_kernel_spmd(nc, in_maps, list(range(NCORES))).results
    return _unpack(res)
